# revision 17
# baseline (speedup 1.0000x reference)
"""Trainium2 Bass kernel for the Cheirality loss layer (v20: fp8 DoubleRow).

Math (per batch b, pixel (y, x); g = grad_dirs, n = normal_flow):
    d1m  = -(g.AV) = V0*g0 + V1*g1 - V2*(x*g0 + y*g1)
    negr = -(nsum - g.BW)
         = -(n0+n1) - O1*g0 + (O0 - O2*x)*g1 - O1*x*(x*g0 + y*g1)
           + (O0*x + O2)*(y*g0) + O0*(y^2*g1)
    out  = mean(gelu(-rho)),  rho = d1m * negr   (exact erf gelu)

Design (v20) — all per-pixel products come from fp8 DoubleRow matmuls:
  * 7 fp8e4m3 basis planes per batch, host-prepared with power-of-2
    scales: G0, G1, XG0=x*g0/64, P2=y*g1/64, NST=(n0+n1)/4,
    YY1=y^2*g1/8192, P0=y*g0/64. Pose coefficients stay on-device in
    the diag stationaries, with (value, residual) split pairs for the
    dominant V2 and O0 coefficients (measured rel err ~2.6e-4).
  * PE: 7 DoubleRow fp8 matmuls per x-slice, accumulating d1m
    (scale 1/8) and negr (scale 1/1024) into separate PSUM banks.
  * Every chunk (and stat piece) is its own CONTIGUOUS DRAM tensor so
    the HBM side streams sequential bursts; chunks/stat pieces are
    spread over the three DMA queues in arrival-need order.
  * Drain: ACT pulls d1m out of PSUM, DVE pulls negr (both bf16, which
    frees the PSUM pair fast), DVE multiplies them at 2x, ACT does
    gelu(scale=-8192) + per-chunk accum columns.
Column-group layout: partition q <-> (batch=q//64, c=q%64); pixel
(x = c + 64*j, y) at free index j*480 + y, NSLICE=10 x-groups.
Reduction: ACT accum -> [128, NCHUNK] partials, host sums in float64.
"""

import numpy as np
import ml_dtypes

import concourse.bacc as bacc
import concourse.bass as bass
import concourse.tile as tile
from concourse import mybir
from concourse.bass_utils import run_bass_kernel_spmd

# Problem geometry (hardcoded per the task contract).
B, H, W = 16, 480, 640
NCORES = 8
BPC = B // NCORES       # 2 batches per core
PHALF = 64              # partitions per batch
NSLICE = 10             # x-groups: x = (q % 64) + 64*j
FS = H                  # 480 free elems per slice
FTOT = NSLICE * FS      # 4800 free elems per partition
FCMAX = 2 * FS
NPLANE = 7              # G0, G1, XG0, P2, NST, YY1, P0
NSTAT = 4 + 3 * NSLICE  # shared: v01, v2c, v2r, nyc; per-slice: og01, o1x, yyp0

F32 = mybir.dt.float32
BF16 = mybir.dt.bfloat16
FP8 = mybir.dt.float8e4
AF = mybir.ActivationFunctionType
DR = mybir.MatmulPerfMode.DoubleRow

CHUNKS = [1, 2, 2, 2, 2, 1]
S0S = [0, 1, 3, 5, 7, 9]
NCHUNK = len(CHUNKS)

# stationary indices
ST_V01, ST_V2C, ST_V2R, ST_NYC = range(4)
def ST_OG01(j): return 4 + 3 * j
def ST_O1X(j): return 5 + 3 * j
def ST_YYP0(j): return 6 + 3 * j

# stat DMA pieces: shared+slice0 / slices 1-4 / slices 5-9
ST_SPLITS = [0, ST_OG01(1), ST_OG01(5), NSTAT]


def _build_kernel(tc, gns_list, stat_list, out):
    nc = tc.nc

    with (
        tc.tile_pool(name="singles", bufs=1) as singles,
        tc.tile_pool(name="ins", bufs=4) as ins,
        tc.tile_pool(name="mids", bufs=3) as mids,
        tc.tile_pool(name="psum", bufs=2, space="PSUM") as psp,
    ):
        stt = singles.tile([128, NSTAT, 2, 128], FP8, name="stt")
        acc = singles.tile([128, NCHUNK], F32, name="acc")

        def gnt_dma(ci, eng):
            FC = CHUNKS[ci] * FS
            t = ins.tile(
                [128, NPLANE * FC], FP8,
                tag=f"gnt{CHUNKS[ci]}", name=f"gnt_{ci}",
            )
            eng.dma_start(out=t, in_=gns_list[ci].ap())
            return t

        def stat_dma(pi, eng):
            a, b = ST_SPLITS[pi], ST_SPLITS[pi + 1]
            eng.dma_start(out=stt[:, a:b], in_=stat_list[pi].ap())

        # DMA plan, arrival-need ordered:
        #   scalar: c0 -> statB(slices 1-4) -> c2 -> c4
        #   sync:   statA(shared+slice0) -> c1 -> c3
        #   swdge:  statC(slices 5-9) -> c5
        gnts = [None] * NCHUNK
        stat_dma(0, nc.sync)
        gnts[0] = gnt_dma(0, nc.scalar)
        stat_dma(2, nc.gpsimd)
        gnts[1] = gnt_dma(1, nc.sync)
        stat_dma(1, nc.scalar)
        gnts[2] = gnt_dma(2, nc.scalar)
        gnts[3] = gnt_dma(3, nc.sync)
        gnts[5] = gnt_dma(5, nc.gpsimd)
        gnts[4] = gnt_dma(4, nc.scalar)

        # PE p-state warm-up spins into the first chunk's PSUM tile (slice 0
        # resets with start=True, so the garbage never escapes) plus an
        # early ACT Gelu table trigger.
        scratch = singles.tile([128, FS], BF16, name="scratch")
        nc.vector.memset(scratch[:, :], 0.0)
        dumm = singles.tile([128, 16], BF16, name="dumm")
        nc.scalar.activation(
            out=dumm, in_=scratch[:, :16], func=AF.Gelu, bias=0.0, scale=-1.0
        )
        ps0 = psp.tile([128, 4, 512], F32, tag="ps", name="ps_0")
        for w in range(4):
            nc.tensor.matmul(
                ps0[:, w % 2, :FS], scratch[:, :128], scratch[:, :FS],
                start=True, stop=True, skip_group_check=True,
            )

        pend = []  # deferred (dnb, ngb, ns, ci) awaiting rho+gelu

        def drain_one():
            dnb, ngb, ns, ci = pend.pop(0)
            rho = mids.tile([128, 2, FS], BF16, tag="rho", name=f"rho_{ci}")[:, :ns]
            nc.vector.tensor_mul(out=rho, in0=ngb, in1=dnb)
            gl = mids.tile([128, 2, FS], BF16, tag="gl", name=f"gl_{ci}")[:, :ns]
            nc.scalar.activation(
                out=gl, in_=rho, func=AF.Gelu, bias=0.0, scale=-8192.0,
                accum_out=acc[:, ci : ci + 1],
            )

        for ci, ns in enumerate(CHUNKS):
            j0 = S0S[ci]
            FC = ns * FS
            gnt = gnts[ci]
            if ci == 0:
                ps = ps0
            else:
                ps = psp.tile([128, 4, 512], F32, tag="ps", name=f"ps_{ci}")

            def mv(a, s):  # moving pair AP: planes [a, a+1], slice s
                return gnt[:, a * FC : (a + 2) * FC].rearrange(
                    "p (c f) -> p c f", c=2
                )[:, :, s * FS : (s + 1) * FS]

            mm = lambda slot, sti, rhs, st, sp: nc.tensor.matmul(
                ps[:, slot, :FS], stt[:, sti], rhs,
                start=st, stop=sp, perf_mode=DR,
            )
            # stationary-major over the chunk's slices to reuse weight loads
            for sti, a, st, sp in (
                (ST_V01, 0, True, False),
                (ST_V2C, 2, False, False),
                (ST_V2R, 2, False, True),
            ):
                for s in range(ns):
                    mm(2 * s, sti, mv(a, s), st, sp)
            for s in range(ns):
                mm(2 * s + 1, ST_OG01(j0 + s), mv(0, s), True, False)
            for s in range(ns):
                mm(2 * s + 1, ST_O1X(j0 + s), mv(2, s), False, False)
            for s in range(ns):
                mm(2 * s + 1, ST_NYC, mv(4, s), False, False)
            for s in range(ns):
                mm(2 * s + 1, ST_YYP0(j0 + s), mv(5, s), False, True)

            # pull both accumulators out of PSUM promptly (frees the ps
            # buffer for chunk ci+2): ACT takes d1m, DVE takes negr
            dnb = mids.tile([128, 2, FS], BF16, tag="dnb", name=f"dnb_{ci}")[:, :ns]
            nc.scalar.activation(
                out=dnb, in_=ps[:, 0 : 2 * ns : 2, :FS], func=AF.Copy
            )
            ngb = mids.tile([128, 2, FS], BF16, tag="ngb", name=f"ngb_{ci}")[:, :ns]
            nc.vector.tensor_scalar_mul(ngb, ps[:, 1 : 2 * ns : 2, :FS], 1.0)

            pend.append((dnb, ngb, ns, ci))
            if len(pend) > 1:
                drain_one()

        while pend:
            drain_one()

        nc.sync.dma_start(out=out.ap(), in_=acc)


def build_bass():
    nc = bacc.Bacc("TRN2", target_bir_lowering=False, debug=False)
    gns_list = [
        nc.dram_tensor(
            f"gns{ci}", [128, NPLANE * CHUNKS[ci] * FS], FP8, kind="ExternalInput"
        )
        for ci in range(NCHUNK)
    ]
    stat_list = [
        nc.dram_tensor(
            f"stat{pi}",
            [128, ST_SPLITS[pi + 1] - ST_SPLITS[pi], 2, 128],
            FP8,
            kind="ExternalInput",
        )
        for pi in range(3)
    ]
    out = nc.dram_tensor("acc_out", [128, NCHUNK], F32, kind="ExternalOutput")
    with tile.TileContext(nc) as tc:
        _build_kernel(tc, gns_list, stat_list, out)
    nc.compile()
    return nc


def _to_plane(a):
    # [H, W] image -> [64, 4800] column-group layout:
    # plane[c, j*480 + y] = a[y, c + 64*j]
    return np.ascontiguousarray(
        a.reshape(H, NSLICE, PHALF).transpose(2, 1, 0).reshape(PHALF, FTOT)
    )


FP8NP = ml_dtypes.float8_e4m3


def _q8(a):
    return np.clip(a, -224.0, 224.0).astype(np.float32).astype(FP8NP)


def make_in_maps(pose, grad_dirs, normal_flow):
    pose = np.asarray(pose, np.float32)
    gd = np.asarray(grad_dirs, np.float32)
    nf = np.asarray(normal_flow, np.float32)

    yr = np.arange(FS, dtype=np.float32)
    yt = np.tile(yr, NSLICE)[None, :]                  # [1, 4800] y per free idx
    xs = np.arange(PHALF, dtype=np.float32)            # x base per partition

    in_maps = []
    for core in range(NCORES):
        b0 = core * BPC
        planes = np.empty((128, NPLANE, FTOT), FP8NP)
        coef = np.zeros((128, NSTAT, 2), np.float64)
        for h in range(BPC):
            bb = b0 + h
            V, O = pose[bb, :3].astype(np.float64), pose[bb, 3:].astype(np.float64)
            rows = slice(h * PHALF, (h + 1) * PHALF)
            g0 = _to_plane(gd[bb, 0])
            g1 = _to_plane(gd[bb, 1])
            nsum = _to_plane(nf[bb, 0] + nf[bb, 1])
            # x per (partition, free idx) in column-group layout
            xg = (xs[:, None] + 64.0 * (np.arange(NSLICE, dtype=np.float32))[None, :])
            xpf = np.repeat(xg, FS, axis=1)            # [64, 4800]
            planes[rows, 0] = _q8(g0)
            planes[rows, 1] = _q8(g1)
            planes[rows, 2] = _q8(xpf * g0 / 64.0)
            planes[rows, 3] = _q8(yt * g1 / 64.0)
            planes[rows, 4] = _q8(nsum / 4.0)
            planes[rows, 5] = _q8(yt * yt * g1 / 8192.0)
            planes[rows, 6] = _q8(yt * g0 / 64.0)

            cf = coef[rows]                            # view [64, NSTAT, 2]
            v2 = -8.0 * V[2]
            v2c = _q8(v2).astype(np.float64)
            yy = 8.0 * O[0]
            yyc = _q8(yy).astype(np.float64)
            cf[:, ST_V01, 0] = V[0] / 8.0
            cf[:, ST_V01, 1] = V[1] / 8.0
            cf[:, ST_V2C, :] = v2c
            cf[:, ST_V2R, :] = v2 - v2c
            cf[:, ST_NYC, 0] = -1.0 / 256.0
            cf[:, ST_NYC, 1] = yyc
            for j in range(NSLICE):
                xj = (xs + 64.0 * j).astype(np.float64)
                cf[:, ST_OG01(j), 0] = -O[1] / 1024.0
                cf[:, ST_OG01(j), 1] = (O[0] - O[2] * xj) / 1024.0
                cf[:, ST_O1X(j), 0] = -O[1] * xj / 16.0
                cf[:, ST_O1X(j), 1] = -O[1] * xj / 16.0
                cf[:, ST_YYP0(j), 0] = yy - yyc
                cf[:, ST_YYP0(j), 1] = (O[0] * xj + O[2]) / 16.0

        # dense diag stationaries from the quantized coefficients
        cq = _q8(coef).astype(np.float32)
        stat = np.zeros((128, NSTAT, 2, 128), np.float32)
        pidx = np.arange(128)
        stat[pidx, :, :, pidx] = cq
        stat8 = stat.astype(FP8NP)

        m = {}
        for ci, ns in enumerate(CHUNKS):
            f0, FC = S0S[ci] * FS, ns * FS
            m[f"gns{ci}"] = np.ascontiguousarray(
                planes[:, :, f0 : f0 + FC].reshape(128, NPLANE * FC)
            )
        for pi in range(3):
            a, b = ST_SPLITS[pi], ST_SPLITS[pi + 1]
            m[f"stat{pi}"] = np.ascontiguousarray(stat8[:, a:b])
        in_maps.append(m)
    return in_maps


_NC_CACHE = None


def _get_nc():
    global _NC_CACHE
    if _NC_CACHE is None:
        _NC_CACHE = build_bass()
    return _NC_CACHE


def kernel(pose, grad_dirs, normal_flow):
    nc = _get_nc()
    in_maps = make_in_maps(pose, grad_dirs, normal_flow)
    res = run_bass_kernel_spmd(nc, in_maps, core_ids=list(range(NCORES)))
    total = 0.0
    for r in res.results:
        total += r["acc_out"].astype(np.float64).sum()
    return np.float32(total / (B * H * W))


# revision 18
# speedup vs baseline: 1.0189x; 1.0189x over previous
"""Trainium2 Bass kernel for the Cheirality loss layer (v20: fp8 DoubleRow).

Math (per batch b, pixel (y, x); g = grad_dirs, n = normal_flow):
    d1m  = -(g.AV) = V0*g0 + V1*g1 - V2*(x*g0 + y*g1)
    negr = -(nsum - g.BW)
         = -(n0+n1) - O1*g0 + (O0 - O2*x)*g1 - O1*x*(x*g0 + y*g1)
           + (O0*x + O2)*(y*g0) + O0*(y^2*g1)
    out  = mean(gelu(-rho)),  rho = d1m * negr   (exact erf gelu)

Design (v20) — all per-pixel products come from fp8 DoubleRow matmuls:
  * 7 fp8e4m3 basis planes per batch, host-prepared with power-of-2
    scales: G0, G1, XG0=x*g0/64, P2=y*g1/64, NST=(n0+n1)/4,
    YY1=y^2*g1/8192, P0=y*g0/64. Pose coefficients stay on-device in
    the diag stationaries, with (value, residual) split pairs for the
    dominant V2 and O0 coefficients (measured rel err ~2.6e-4).
  * PE: 7 DoubleRow fp8 matmuls per x-slice, accumulating d1m
    (scale 1/8) and negr (scale 1/1024) into separate PSUM banks.
  * Every chunk (and stat piece) is its own CONTIGUOUS DRAM tensor so
    the HBM side streams sequential bursts; chunks/stat pieces are
    spread over the three DMA queues in arrival-need order.
  * Drain: ACT pulls d1m out of PSUM, DVE pulls negr (both bf16, which
    frees the PSUM pair fast), DVE multiplies them at 2x, ACT does
    gelu(scale=-8192) + per-chunk accum columns.
Column-group layout: partition q <-> (batch=q//64, c=q%64); pixel
(x = c + 64*j, y) at free index j*480 + y, NSLICE=10 x-groups.
Reduction: ACT accum -> [128, NCHUNK] partials, host sums in float64.
"""

import numpy as np
import ml_dtypes

import concourse.bacc as bacc
import concourse.bass as bass
import concourse.tile as tile
from concourse import mybir
from concourse.bass_utils import run_bass_kernel_spmd

# Problem geometry (hardcoded per the task contract).
B, H, W = 16, 480, 640
NCORES = 8
BPC = B // NCORES       # 2 batches per core
PHALF = 64              # partitions per batch
NSLICE = 10             # x-groups: x = (q % 64) + 64*j
FS = H                  # 480 free elems per slice
FTOT = NSLICE * FS      # 4800 free elems per partition
FCMAX = 2 * FS
NPLANE = 7              # G0, G1, XG0, P2, NST, YY1, P0
NSTAT = 4 + 3 * NSLICE  # shared: v01, v2c, v2r, nyc; per-slice: og01, o1x, yyp0

F32 = mybir.dt.float32
BF16 = mybir.dt.bfloat16
FP8 = mybir.dt.float8e4
AF = mybir.ActivationFunctionType
DR = mybir.MatmulPerfMode.DoubleRow

CHUNKS = [1, 2, 2, 2, 2, 1]
S0S = [0, 1, 3, 5, 7, 9]
NCHUNK = len(CHUNKS)

# stationary indices
ST_V01, ST_V2C, ST_V2R, ST_NYC = range(4)
def ST_OG01(j): return 4 + 3 * j
def ST_O1X(j): return 5 + 3 * j
def ST_YYP0(j): return 6 + 3 * j

# stat DMA pieces: shared+slice0 / slices 1-4 / slices 5-9
ST_SPLITS = [0, ST_OG01(1), ST_OG01(5), NSTAT]


def _build_kernel(tc, gns_list, stat_list, out):
    nc = tc.nc

    with (
        tc.tile_pool(name="singles", bufs=1) as singles,
        tc.tile_pool(name="ins", bufs=4) as ins,
        tc.tile_pool(name="mids", bufs=3) as mids,
        tc.tile_pool(name="psum", bufs=2, space="PSUM") as psp,
    ):
        stt = singles.tile([128, NSTAT, 2, 128], FP8, name="stt")
        acc = singles.tile([128, NCHUNK], F32, name="acc")

        def gnt_dma(ci, split=True):
            # split each chunk into two free-dim byte-halves: the fast
            # scalar HWDGE queue takes one, the gpsimd SWDGE queue the
            # other, so consecutive chunks arrive at the combined rate
            FC = CHUNKS[ci] * FS
            nb = NPLANE * FC
            t = ins.tile(
                [128, nb], FP8, tag=f"gnt{CHUNKS[ci]}", name=f"gnt_{ci}",
            )
            src = gns_list[ci].ap()
            if split:
                h = (nb // 2 + 63) & ~63
                nc.scalar.dma_start(out=t[:, :h], in_=src[:, :h])
                nc.gpsimd.dma_start(out=t[:, h:], in_=src[:, h:])
            else:
                nc.scalar.dma_start(out=t, in_=src)
            return t

        def stat_dma(pi, eng):
            a, b = ST_SPLITS[pi], ST_SPLITS[pi + 1]
            eng.dma_start(out=stt[:, a:b], in_=stat_list[pi].ap())

        # DMA plan, arrival-need ordered: stats ride the (slow) sync queue
        # in slice order; chunk 0 goes whole on scalar; later chunks are
        # half-split scalar+swdge.
        stat_dma(0, nc.sync)
        gnts = [gnt_dma(0, split=False)]
        stat_dma(1, nc.sync)
        gnts.append(gnt_dma(1))
        stat_dma(2, nc.sync)
        for ci in range(2, NCHUNK):
            gnts.append(gnt_dma(ci))

        # PE p-state warm-up spins into the first chunk's PSUM tile (slice 0
        # resets with start=True, so the garbage never escapes) plus an
        # early ACT Gelu table trigger.
        scratch = singles.tile([128, FS], BF16, name="scratch")
        nc.vector.memset(scratch[:, :], 0.0)
        dumm = singles.tile([128, 16], BF16, name="dumm")
        nc.scalar.activation(
            out=dumm, in_=scratch[:, :16], func=AF.Gelu, bias=0.0, scale=-1.0
        )
        ps0 = psp.tile([128, 4, 512], F32, tag="ps", name="ps_0")
        for w in range(4):
            nc.tensor.matmul(
                ps0[:, w % 2, :FS], scratch[:, :128], scratch[:, :FS],
                start=True, stop=True, skip_group_check=True,
            )

        pend = []  # deferred (dnb, ngb, ns, ci) awaiting rho+gelu

        def drain_one():
            dnb, ngb, ns, ci = pend.pop(0)
            rho = mids.tile([128, 2, FS], BF16, tag="rho", name=f"rho_{ci}")[:, :ns]
            nc.vector.tensor_mul(out=rho, in0=ngb, in1=dnb)
            gl = mids.tile([128, 2, FS], BF16, tag="gl", name=f"gl_{ci}")[:, :ns]
            nc.scalar.activation(
                out=gl, in_=rho, func=AF.Gelu, bias=0.0, scale=-8192.0,
                accum_out=acc[:, ci : ci + 1],
            )

        for ci, ns in enumerate(CHUNKS):
            j0 = S0S[ci]
            FC = ns * FS
            gnt = gnts[ci]
            if ci == 0:
                ps = ps0
            else:
                ps = psp.tile([128, 4, 512], F32, tag="ps", name=f"ps_{ci}")

            def mv(a, s):  # moving pair AP: planes [a, a+1], slice s
                return gnt[:, a * FC : (a + 2) * FC].rearrange(
                    "p (c f) -> p c f", c=2
                )[:, :, s * FS : (s + 1) * FS]

            mm = lambda slot, sti, rhs, st, sp: nc.tensor.matmul(
                ps[:, slot, :FS], stt[:, sti], rhs,
                start=st, stop=sp, perf_mode=DR,
            )
            # stationary-major over the chunk's slices to reuse weight loads
            for sti, a, st, sp in (
                (ST_V01, 0, True, False),
                (ST_V2C, 2, False, False),
                (ST_V2R, 2, False, True),
            ):
                for s in range(ns):
                    mm(2 * s, sti, mv(a, s), st, sp)
            for s in range(ns):
                mm(2 * s + 1, ST_OG01(j0 + s), mv(0, s), True, False)
            for s in range(ns):
                mm(2 * s + 1, ST_O1X(j0 + s), mv(2, s), False, False)
            for s in range(ns):
                mm(2 * s + 1, ST_NYC, mv(4, s), False, False)
            for s in range(ns):
                mm(2 * s + 1, ST_YYP0(j0 + s), mv(5, s), False, True)

            # pull both accumulators out of PSUM promptly (frees the ps
            # buffer for chunk ci+2): ACT takes d1m, DVE takes negr
            dnb = mids.tile([128, 2, FS], BF16, tag="dnb", name=f"dnb_{ci}")[:, :ns]
            nc.scalar.activation(
                out=dnb, in_=ps[:, 0 : 2 * ns : 2, :FS], func=AF.Copy
            )
            ngb = mids.tile([128, 2, FS], BF16, tag="ngb", name=f"ngb_{ci}")[:, :ns]
            nc.vector.tensor_scalar_mul(ngb, ps[:, 1 : 2 * ns : 2, :FS], 1.0)

            pend.append((dnb, ngb, ns, ci))
            if len(pend) > 1:
                drain_one()

        while pend:
            drain_one()

        nc.sync.dma_start(out=out.ap(), in_=acc)


def build_bass():
    nc = bacc.Bacc("TRN2", target_bir_lowering=False, debug=False)
    gns_list = [
        nc.dram_tensor(
            f"gns{ci}", [128, NPLANE * CHUNKS[ci] * FS], FP8, kind="ExternalInput"
        )
        for ci in range(NCHUNK)
    ]
    stat_list = [
        nc.dram_tensor(
            f"stat{pi}",
            [128, ST_SPLITS[pi + 1] - ST_SPLITS[pi], 2, 128],
            FP8,
            kind="ExternalInput",
        )
        for pi in range(3)
    ]
    out = nc.dram_tensor("acc_out", [128, NCHUNK], F32, kind="ExternalOutput")
    with tile.TileContext(nc) as tc:
        _build_kernel(tc, gns_list, stat_list, out)
    nc.compile()
    return nc


def _to_plane(a):
    # [H, W] image -> [64, 4800] column-group layout:
    # plane[c, j*480 + y] = a[y, c + 64*j]
    return np.ascontiguousarray(
        a.reshape(H, NSLICE, PHALF).transpose(2, 1, 0).reshape(PHALF, FTOT)
    )


FP8NP = ml_dtypes.float8_e4m3


def _q8(a):
    return np.clip(a, -224.0, 224.0).astype(np.float32).astype(FP8NP)


def make_in_maps(pose, grad_dirs, normal_flow):
    pose = np.asarray(pose, np.float32)
    gd = np.asarray(grad_dirs, np.float32)
    nf = np.asarray(normal_flow, np.float32)

    yr = np.arange(FS, dtype=np.float32)
    yt = np.tile(yr, NSLICE)[None, :]                  # [1, 4800] y per free idx
    xs = np.arange(PHALF, dtype=np.float32)            # x base per partition

    in_maps = []
    for core in range(NCORES):
        b0 = core * BPC
        planes = np.empty((128, NPLANE, FTOT), FP8NP)
        coef = np.zeros((128, NSTAT, 2), np.float64)
        for h in range(BPC):
            bb = b0 + h
            V, O = pose[bb, :3].astype(np.float64), pose[bb, 3:].astype(np.float64)
            rows = slice(h * PHALF, (h + 1) * PHALF)
            g0 = _to_plane(gd[bb, 0])
            g1 = _to_plane(gd[bb, 1])
            nsum = _to_plane(nf[bb, 0] + nf[bb, 1])
            # x per (partition, free idx) in column-group layout
            xg = (xs[:, None] + 64.0 * (np.arange(NSLICE, dtype=np.float32))[None, :])
            xpf = np.repeat(xg, FS, axis=1)            # [64, 4800]
            planes[rows, 0] = _q8(g0)
            planes[rows, 1] = _q8(g1)
            planes[rows, 2] = _q8(xpf * g0 / 64.0)
            planes[rows, 3] = _q8(yt * g1 / 64.0)
            planes[rows, 4] = _q8(nsum / 4.0)
            planes[rows, 5] = _q8(yt * yt * g1 / 8192.0)
            planes[rows, 6] = _q8(yt * g0 / 64.0)

            cf = coef[rows]                            # view [64, NSTAT, 2]
            v2 = -8.0 * V[2]
            v2c = _q8(v2).astype(np.float64)
            yy = 8.0 * O[0]
            yyc = _q8(yy).astype(np.float64)
            cf[:, ST_V01, 0] = V[0] / 8.0
            cf[:, ST_V01, 1] = V[1] / 8.0
            cf[:, ST_V2C, :] = v2c
            cf[:, ST_V2R, :] = v2 - v2c
            cf[:, ST_NYC, 0] = -1.0 / 256.0
            cf[:, ST_NYC, 1] = yyc
            for j in range(NSLICE):
                xj = (xs + 64.0 * j).astype(np.float64)
                cf[:, ST_OG01(j), 0] = -O[1] / 1024.0
                cf[:, ST_OG01(j), 1] = (O[0] - O[2] * xj) / 1024.0
                cf[:, ST_O1X(j), 0] = -O[1] * xj / 16.0
                cf[:, ST_O1X(j), 1] = -O[1] * xj / 16.0
                cf[:, ST_YYP0(j), 0] = yy - yyc
                cf[:, ST_YYP0(j), 1] = (O[0] * xj + O[2]) / 16.0

        # dense diag stationaries from the quantized coefficients
        cq = _q8(coef).astype(np.float32)
        stat = np.zeros((128, NSTAT, 2, 128), np.float32)
        pidx = np.arange(128)
        stat[pidx, :, :, pidx] = cq
        stat8 = stat.astype(FP8NP)

        m = {}
        for ci, ns in enumerate(CHUNKS):
            f0, FC = S0S[ci] * FS, ns * FS
            m[f"gns{ci}"] = np.ascontiguousarray(
                planes[:, :, f0 : f0 + FC].reshape(128, NPLANE * FC)
            )
        for pi in range(3):
            a, b = ST_SPLITS[pi], ST_SPLITS[pi + 1]
            m[f"stat{pi}"] = np.ascontiguousarray(stat8[:, a:b])
        in_maps.append(m)
    return in_maps


_NC_CACHE = None


def _get_nc():
    global _NC_CACHE
    if _NC_CACHE is None:
        _NC_CACHE = build_bass()
    return _NC_CACHE


def kernel(pose, grad_dirs, normal_flow):
    nc = _get_nc()
    in_maps = make_in_maps(pose, grad_dirs, normal_flow)
    res = run_bass_kernel_spmd(nc, in_maps, core_ids=list(range(NCORES)))
    total = 0.0
    for r in res.results:
        total += r["acc_out"].astype(np.float64).sum()
    return np.float32(total / (B * H * W))
